# revision 1
# baseline (speedup 1.0000x reference)
"""GAT (3-layer, 6/6/1 heads) + MLP classifier on Trainium2, 8 NeuronCores.

Sharding: destination-node partition. Each core owns N/8 = 6250 dst nodes and
all edges into them (host sorts edges by dst). Per layer: each core projects
features for its own nodes (bf16 matmuls); two AllGathers replicate the
projected table (lower halves of every core's shard first — launched
mid-projection so it hides under compute — then upper halves, into row slices
of one shared table). Each core then aggregates messages for its dst tiles:
  - self-loops form a dedicated full chunk per tile whose rows are the tile's
    own shard rows (plain contiguous DMA, identity mask),
  - remaining source rows arrive via per-chunk indirect-DMA gathers,
  - per-edge alpha_dst comes from PE (transpose the 0/1 dst mask, matmul
    against the tile's alpha_dst values) instead of per-edge gathers,
  - attention softmax runs as wide batched vector ops across all chunks,
  - bf16 mask matmuls scatter-add messages; the softmax denominator rides
    along as extra matmul columns.
"""

import sys

sys.path.insert(0, "/opt/trn_rl_repo")

import numpy as np

from concourse import bass, mybir, tile, bacc
from concourse import bass_utils

P = 128
N, E, F_IN, C, H, N_CLS = 50000, 200000, 256, 128, 6, 40
HC = H * C  # 768
NCORES = 8
BN_EPS = 1e-5
NEG_SLOPE = 0.2

F32 = mybir.dt.float32
BF16 = mybir.dt.bfloat16
TDT = BF16
I32 = mybir.dt.int32
AX = mybir.AxisListType
ALU = mybir.AluOpType
ACTF = mybir.ActivationFunctionType


# ---------------------------------------------------------------- host prep
def _edge_arrays(edge_index, n_nodes, ncores):
    """v3: edges sorted by dst, chunks per dst tile, single table row space."""
    npc = n_nodes // ncores
    tpc = (npc + P - 1) // P
    src = np.concatenate([edge_index[0], np.arange(n_nodes, dtype=np.int32)])
    dst = np.concatenate([edge_index[1], np.arange(n_nodes, dtype=np.int32)])
    order = np.argsort(dst, kind="stable")
    src_s = src[order].astype(np.int64)
    dst_s = dst[order].astype(np.int64)

    core = dst_s // npc
    t = (dst_s % npc) // P
    block = core * tpc + t
    counts = np.bincount(block, minlength=ncores * tpc).reshape(ncores, tpc)
    cpt = np.maximum(1, -(-counts.max(axis=0) // P))
    col_off = np.concatenate([[0], np.cumsum(cpt)])[:-1]
    n_chunks = int(cpt.sum())

    starts = np.searchsorted(block, np.arange(ncores * tpc))
    j = np.arange(len(dst_s)) - starts[block]
    col = col_off[t] + j // P
    part = j % P

    srcT = np.zeros((ncores, P, n_chunks), np.int32)
    dstlocT = np.full((ncores, P, n_chunks), 200.0, np.float32)
    dstownT = np.zeros((ncores, P, n_chunks), np.int32)
    srcT[core, part, col] = src_s
    dstlocT[core, part, col] = (dst_s - core * npc - t * P).astype(np.float32)
    dstownT[core, part, col] = dst_s - core * npc
    return srcT, dstlocT, dstownT, n_chunks, cpt


def _wext(Wnp, a_s, a_d):
    heads, cout = a_s.shape
    A_s = np.zeros((heads * cout, heads), np.float32)
    A_d = np.zeros((heads * cout, heads), np.float32)
    for h in range(heads):
        A_s[h * cout : (h + 1) * cout, h] = a_s[h]
        A_d[h * cout : (h + 1) * cout, h] = a_d[h]
    return np.concatenate([Wnp, Wnp @ A_s, Wnp @ A_d], axis=1).astype(np.float32)


def _bn_cols(g, be, b):
    inv = 1.0 / np.sqrt(1.0 + BN_EPS)
    scale = (g * inv).astype(np.float32)
    shift = (b * scale + be).astype(np.float32)
    nh = len(g) // P
    return scale.reshape(nh, P).T.copy(), shift.reshape(nh, P).T.copy()


# ------------------------------------------------------------- bass program
def _build(n_chunks, cpt, n_nodes=N, ncores=NCORES):
    npc = n_nodes // ncores
    tpc = (npc + P - 1) // P
    coff = np.concatenate([[0], np.cumsum(cpt)])
    nc = bacc.Bacc("TRN2", target_bir_lowering=False, debug=False,
                   num_devices=ncores)

    def din(name, shape, dt=F32):
        return nc.dram_tensor(name, shape, dt, kind="ExternalInput").ap()

    xT = din("xT", [F_IN, npc], BF16)
    srcT = din("srcT", [P, n_chunks], I32)
    dstlocT = din("dstlocT", [P, n_chunks])
    dstownT = din("dstownT", [P, n_chunks], I32)
    W1e = din("W1e", [F_IN, HC + 12], BF16)
    W2e = din("W2e", [HC, HC + 12], BF16)
    W3e = din("W3e", [HC, C + 2], BF16)
    sc1 = din("sc1", [P, H]); sh1 = din("sh1", [P, H])
    sc2 = din("sc2", [P, H]); sh2 = din("sh2", [P, H])
    sc3 = din("sc3", [P, 1]); sh3 = din("sh3", [P, 1])
    Wc1 = din("Wc1", [C, C // 2])
    bc1 = din("bc1", [C // 2, 1])
    Wc2 = din("Wc2", [C // 2, N_CLS])
    bc2 = din("bc2", [N_CLS, 1])
    out = nc.dram_tensor("out", [npc, N_CLS], F32, kind="ExternalOutput").ap()

    rg = [list(range(ncores))]

    with tile.TileContext(nc) as tc:
        from contextlib import ExitStack
        ctx = ExitStack()
        cons = ctx.enter_context(tc.tile_pool(name="cons", bufs=1))
        dram = ctx.enter_context(tc.tile_pool(name="dram", bufs=1, space="DRAM"))
        big = ctx.enter_context(tc.tile_pool(name="big", bufs=2))
        sb = ctx.enter_context(tc.tile_pool(name="sb", bufs=3))
        ep = ctx.enter_context(tc.tile_pool(name="ep", bufs=2))
        ps_agg = ctx.enter_context(tc.tile_pool(name="ps_agg", bufs=2, space="PSUM"))
        ps_tp = ctx.enter_context(tc.tile_pool(name="ps_tp", bufs=1, space="PSUM"))
        ps_ad = ctx.enter_context(tc.tile_pool(name="ps_ad", bufs=1, space="PSUM"))
        ps_pj = ctx.enter_context(tc.tile_pool(name="ps_pj", bufs=1, space="PSUM"))

        def load_const(ap_in, shape, dt=F32, name="c"):
            t = cons.tile(shape, dt, name=name)
            nc.sync.dma_start(out=t[:], in_=ap_in)
            return t

        w1_sb = [load_const(W1e[k * P:(k + 1) * P, :], [P, HC + 12], BF16,
                            name=f"w1_{k}") for k in range(F_IN // P)]
        w2_sb = [load_const(W2e[k * P:(k + 1) * P, :], [P, HC + 12], BF16,
                            name=f"w2_{k}") for k in range(HC // P)]
        w3_sb = [load_const(W3e[k * P:(k + 1) * P, :], [P, C + 2], BF16,
                            name=f"w3_{k}") for k in range(HC // P)]
        sc1_sb = load_const(sc1[:, :], [P, H], name="sc1")
        sh1_sb = load_const(sh1[:, :], [P, H], name="sh1")
        sc2_sb = load_const(sc2[:, :], [P, H], name="sc2")
        sh2_sb = load_const(sh2[:, :], [P, H], name="sh2")
        sc3_sb = load_const(sc3[:, :], [P, 1], name="sc3")
        sh3_sb = load_const(sh3[:, :], [P, 1], name="sh3")
        wc1_sb = load_const(Wc1[:, :], [C, C // 2], name="wc1")
        bc1_sb = load_const(bc1[:, :], [C // 2, 1], name="bc1")
        wc2_sb = load_const(Wc2[:, :], [C // 2, N_CLS], name="wc2")
        bc2_sb = load_const(bc2[:, :], [N_CLS, 1], name="bc2")
        src_sb = load_const(srcT[:, :], [P, n_chunks], I32, name="srcsb")
        dloc_sb = load_const(dstlocT[:, :], [P, n_chunks], name="dlocsb")
        down_sb = load_const(dstownT[:, :], [P, n_chunks], I32, name="downsb")

        iota_i = cons.tile([P, P], I32, name="iota_i")
        nc.gpsimd.iota(iota_i[:], pattern=[[1, P]], base=0, channel_multiplier=0)
        iota_f = cons.tile([P, P], F32, name="iota_f")
        nc.vector.tensor_copy(out=iota_f[:], in_=iota_i[:])
        ident = cons.tile([P, P], F32, name="ident")
        from concourse.masks import make_identity
        make_identity(nc, ident[:])
        ident_bf = cons.tile([P, P], BF16, name="ident_bf")
        nc.vector.tensor_copy(out=ident_bf[:], in_=ident[:])

        # ---- DRAM intermediates
        shard = [dram.tile([npc, HC + 6], TDT, name="shard1"),
                 dram.tile([npc, HC + 6], TDT, name="shard2"),
                 dram.tile([npc, C + 1], TDT, name="shard3")]
        shard_ad = [dram.tile([npc, H], TDT, name="shard_ad1"),
                    dram.tile([npc, H], TDT, name="shard_ad2"),
                    dram.tile([npc, 1], TDT, name="shard_ad3")]
        table = [dram.tile([n_nodes, HC + 6], TDT, addr_space="Shared",
                           name="table1"),
                 dram.tile([n_nodes, HC + 6], TDT, addr_space="Shared",
                           name="table2"),
                 dram.tile([n_nodes, C + 1], TDT, addr_space="Shared",
                           name="table3")]

        def nt_of(t):
            return min(P, npc - t * P)

        def project(li, t, lhs_tiles):
            nt = nt_of(t)
            w_sb = [w1_sb, w2_sb, w3_sb][li]
            wid = HC + 12 if li < 2 else C + 2
            hwid = HC + 6 if li < 2 else C + 1
            nk = len(w_sb)
            hp = ep.tile([P, hwid], TDT, tag="hp")
            hpad = ep.tile([P, wid - hwid], TDT, tag="hpad")
            if li < 2:
                pA = ps_pj.tile([P, 512], F32, tag="pjA")
                pB = ps_pj.tile([P, wid - 512], F32, tag="pjB")
                for k in range(nk):
                    nc.tensor.matmul(out=pA[:nt, :], lhsT=lhs_tiles[k][:, :nt],
                                     rhs=w_sb[k][:, 0:512],
                                     start=(k == 0), stop=(k == nk - 1))
                    nc.tensor.matmul(out=pB[:nt, :], lhsT=lhs_tiles[k][:, :nt],
                                     rhs=w_sb[k][:, 512:wid],
                                     start=(k == 0), stop=(k == nk - 1))
                nc.scalar.activation(out=hp[:nt, 0:512], in_=pA[:nt, :], func=ACTF.Copy)
                nc.scalar.activation(out=hp[:nt, 512:hwid],
                                     in_=pB[:nt, 0:hwid - 512], func=ACTF.Copy)
                nc.vector.tensor_copy(out=hpad[:nt, :],
                                      in_=pB[:nt, hwid - 512:wid - 512])
            else:
                pA = ps_pj.tile([P, wid], F32, tag="pjA")
                for k in range(nk):
                    nc.tensor.matmul(out=pA[:nt, :], lhsT=lhs_tiles[k][:, :nt],
                                     rhs=w_sb[k][:, :],
                                     start=(k == 0), stop=(k == nk - 1))
                nc.scalar.activation(out=hp[:nt, :], in_=pA[:nt, 0:hwid],
                                     func=ACTF.Copy)
                nc.vector.tensor_copy(out=hpad[:nt, :], in_=pA[:nt, hwid:wid])
            r0 = t * P
            nc.sync.dma_start(out=shard[li][r0:r0 + nt, :], in_=hp[:nt, :])
            nc.sync.dma_start(out=shard_ad[li][r0:r0 + nt, :], in_=hpad[:nt, :])

        def allgather(li):
            nc.gpsimd.collective_compute(
                "AllGather", ALU.bypass, replica_groups=rg,
                ins=[shard[li][:, :].opt()], outs=[table[li][:, :].opt()])

        def aggregate(li, t, nheads):
            nt = nt_of(t)
            hw = C * nheads
            gw = hw + nheads          # row width (h | alpha_src)
            c0 = int(coff[t])
            K = int(cpt[t])
            hs = big.tile([P, K * gw], TDT, tag="hs")
            hs3 = hs[:].rearrange("p (k g) -> p k g", k=K)
            for k in range(K):
                cc = c0 + k
                nc.gpsimd.indirect_dma_start(
                    out=hs[:, k * gw:(k + 1) * gw], out_offset=None,
                    in_=table[li][:, :],
                    in_offset=bass.IndirectOffsetOnAxis(ap=src_sb[:, cc:cc + 1],
                                                        axis=0))
            mask = big.tile([P, K * P], TDT, tag="mask")
            nc.vector.tensor_tensor(
                out=mask[:].rearrange("p (k q) -> p k q", k=K),
                in0=dloc_sb[:, c0:c0 + K].unsqueeze(2).broadcast_to([P, K, P]),
                in1=iota_f[:].unsqueeze(1).broadcast_to([P, K, P]),
                op=ALU.is_equal)
            # per-edge alpha_dst via PE: transpose mask, matmul against the
            # tile's alpha_dst values (replaces 300 tiny gathers per layer)
            adt = ep.tile([P, nheads], TDT, tag="adt")
            if nt < P:
                nc.vector.memset(adt[:], 0.0)
            nc.sync.dma_start(out=adt[:nt, :],
                              in_=shard_ad[li][t * P:t * P + nt, :])
            ad_ps = ps_ad.tile([P, K * nheads], F32, tag="adp")
            for k in range(K):
                mtp = ps_tp.tile([P, P], BF16, tag="tp")
                nc.tensor.transpose(out=mtp[:], in_=mask[:, k * P:(k + 1) * P],
                                    identity=ident_bf[:])
                mts = sb.tile([P, P], BF16, tag="mts")
                nc.vector.tensor_copy(out=mts[:], in_=mtp[:])
                nc.tensor.matmul(out=ad_ps[:, k * nheads:(k + 1) * nheads],
                                 lhsT=mts[:], rhs=adt[:], start=True, stop=True)
            # attention logits + leaky relu + exp, batched over all chunks
            ee = sb.tile([P, K * nheads], F32, tag="ee")
            nc.vector.tensor_tensor(
                out=ee[:].rearrange("p (k h) -> p k h", k=K),
                in0=hs3[:, :, hw:gw],
                in1=ad_ps[:].rearrange("p (k h) -> p k h", k=K), op=ALU.add)
            lk = sb.tile([P, K * nheads], F32, tag="lk")
            nc.vector.tensor_scalar_mul(lk[:], ee[:], NEG_SLOPE)
            nc.vector.tensor_tensor(out=ee[:], in0=ee[:], in1=lk[:], op=ALU.max)
            w = sb.tile([P, K * nheads], TDT, tag="w")
            nc.scalar.activation(out=w[:], in_=ee[:], func=ACTF.Exp)
            w3 = w[:].rearrange("p (k h) -> p k h", k=K)
            msg = big.tile([P, K * gw], TDT, tag="msg")
            msg3 = msg[:].rearrange("p (k g) -> p k g", k=K)
            nc.vector.tensor_tensor(
                out=msg3[:, :, 0:hw].rearrange("p k (h c) -> p k h c", h=nheads),
                in0=hs3[:, :, 0:hw].rearrange("p k (h c) -> p k h c", h=nheads),
                in1=w3.unsqueeze(3).broadcast_to([P, K, nheads, C]),
                op=ALU.mult)
            nc.vector.tensor_copy(out=msg3[:, :, hw:gw], in_=w3)
            if nheads > 1:
                pA = ps_agg.tile([P, 512], F32, tag="agA")
                pB = ps_agg.tile([P, hw + nheads - 512], F32, tag="agB")
            else:
                pA = ps_agg.tile([P, hw + 1], F32, tag="agA")
                pB = None
            for k in range(K):
                st, sp = (k == 0), (k == K - 1)
                lhsT = mask[:, k * P:(k + 1) * P]
                if nheads > 1:
                    nc.tensor.matmul(out=pA[:, :], lhsT=lhsT,
                                     rhs=msg[:, k * gw:k * gw + 512],
                                     start=st, stop=sp)
                    nc.tensor.matmul(out=pB[:, :], lhsT=lhsT,
                                     rhs=msg[:, k * gw + 512:(k + 1) * gw],
                                     start=st, stop=sp)
                else:
                    nc.tensor.matmul(out=pA[:, :], lhsT=lhsT,
                                     rhs=msg[:, k * gw:(k + 1) * gw],
                                     start=st, stop=sp)
            return pA, pB

        # ================= Layer 1 projection (from input xT)
        for t in range(tpc):
            nt = nt_of(t)
            lhs = []
            for k in range(F_IN // P):
                lt = sb.tile([P, P], BF16, tag="xlhs")
                nc.sync.dma_start(out=lt[:, :nt],
                                  in_=xT[k * P:(k + 1) * P, t * P:t * P + nt])
                lhs.append(lt)
            project(0, t, lhs)
        allgather(0)

        # ============ Layers 1,2 aggregation (+ fused next-layer projection)
        for li in range(2):
            sc_sb = [sc1_sb, sc2_sb][li]
            sh_sb = [sh1_sb, sh2_sb][li]
            for t in range(tpc):
                nt = nt_of(t)
                pA, pB = aggregate(li, t, H)
                recip = ep.tile([P, H], F32, tag="recip")
                nc.vector.reciprocal(out=recip[:], in_=pB[:, 256:262])
                agg = ep.tile([P, HC], BF16, tag="agg")
                nc.vector.tensor_tensor(
                    out=agg[:, 0:512].rearrange("p (h c) -> p h c", h=4),
                    in0=pA[:].rearrange("p (h c) -> p h c", h=4),
                    in1=recip[:, 0:4].unsqueeze(2).broadcast_to([P, 4, C]),
                    op=ALU.mult)
                nc.vector.tensor_tensor(
                    out=agg[:, 512:768].rearrange("p (h c) -> p h c", h=2),
                    in0=pB[:, 0:256].rearrange("p (h c) -> p h c", h=2),
                    in1=recip[:, 4:6].unsqueeze(2).broadcast_to([P, 2, C]),
                    op=ALU.mult)
                outT = ep.tile([P, HC], BF16, tag="outT")
                for h in range(H):
                    ptp = ps_tp.tile([P, P], BF16, tag="tp")
                    nc.tensor.transpose(out=ptp[:], in_=agg[:, h * C:(h + 1) * C],
                                        identity=ident_bf[:])
                    nc.scalar.activation(out=outT[:, h * C:(h + 1) * C], in_=ptp[:],
                                         func=ACTF.Relu, bias=sh_sb[:, h:h + 1],
                                         scale=sc_sb[:, h:h + 1])
                lhs = [outT[:, k * P:(k + 1) * P] for k in range(HC // P)]
                project(li + 1, t, lhs)
            allgather(li + 1)

        # ================= Layer 3 aggregation + classifier + log_softmax
        for t in range(tpc):
            nt = nt_of(t)
            pA, _ = aggregate(2, t, 1)
            recip = ep.tile([P, 1], F32, tag="recip3")
            nc.vector.reciprocal(out=recip[:], in_=pA[:, C:C + 1])
            agg = ep.tile([P, C], F32, tag="agg3")
            nc.vector.tensor_scalar_mul(agg[:], pA[:, 0:C], recip[:, 0:1])
            ptp = ps_tp.tile([P, P], F32, tag="tp")
            nc.tensor.transpose(out=ptp[:], in_=agg[:], identity=ident[:])
            y3 = ep.tile([P, P], F32, tag="y3")
            nc.vector.tensor_scalar(out=y3[:], in0=ptp[:], scalar1=sc3_sb[:, 0:1],
                                    scalar2=sh3_sb[:, 0:1], op0=ALU.mult, op1=ALU.add)
            z1p = ps_pj.tile([C // 2, P], F32, tag="pjA")
            nc.tensor.matmul(out=z1p[:, :nt], lhsT=wc1_sb[:], rhs=y3[:, :nt],
                             start=True, stop=True)
            z1 = ep.tile([C // 2, P], F32, tag="z1")
            nc.scalar.activation(out=z1[:, :nt], in_=z1p[:, :nt], func=ACTF.Relu,
                                 bias=bc1_sb[:, 0:1])
            lgp = ps_pj.tile([N_CLS, P], F32, tag="pjB")
            nc.tensor.matmul(out=lgp[:, :nt], lhsT=wc2_sb[:], rhs=z1[:, :nt],
                             start=True, stop=True)
            lgb = ep.tile([N_CLS, P], F32, tag="lgb")
            nc.vector.tensor_scalar(out=lgb[:, :nt], in0=lgp[:, :nt],
                                    scalar1=bc2_sb[:, 0:1], scalar2=None,
                                    op0=ALU.add)
            ptp2 = ps_tp.tile([P, N_CLS], F32, tag="tp")
            nc.tensor.transpose(out=ptp2[:nt, :], in_=lgb[:, :nt],
                                identity=ident[:N_CLS, :N_CLS])
            mx = ep.tile([P, 1], F32, tag="mx")
            nc.vector.reduce_max(out=mx[:nt, :], in_=ptp2[:nt, :], axis=AX.X)
            xs = ep.tile([P, N_CLS], F32, tag="xs")
            nc.vector.tensor_scalar(out=xs[:nt, :], in0=ptp2[:nt, :],
                                    scalar1=mx[:nt, 0:1], scalar2=None,
                                    op0=ALU.subtract)
            ex = ep.tile([P, N_CLS], F32, tag="ex")
            ssum = ep.tile([P, 1], F32, tag="ssum")
            nc.scalar.activation(out=ex[:nt, :], in_=xs[:nt, :], func=ACTF.Exp,
                                 accum_out=ssum[:nt, 0:1])
            lns = ep.tile([P, 1], F32, tag="lns")
            nc.scalar.activation(out=lns[:nt, :], in_=ssum[:nt, :], func=ACTF.Ln)
            fin = ep.tile([P, N_CLS], F32, tag="fin")
            nc.vector.tensor_scalar(out=fin[:nt, :], in0=xs[:nt, :],
                                    scalar1=lns[:nt, 0:1], scalar2=None,
                                    op0=ALU.subtract)
            nc.sync.dma_start(out=out[t * P:t * P + nt, :], in_=fin[:nt, :])
        ctx.close()

    nc.compile()
    return nc


_CACHE = {}


def _get_program(edge_index_bytes, edge_index):
    key = edge_index_bytes
    if key not in _CACHE:
        srcT, dstlocT, dstownT, n_chunks, cpt = _edge_arrays(edge_index, N, NCORES)
        nc = _build(n_chunks, cpt)
        _CACHE[key] = (nc, srcT, dstlocT, dstownT)
    return _CACHE[key]


def prepare(inputs):
    """Returns (nc, in_maps) for the given full inputs."""
    x = np.asarray(inputs["x"], np.float32)
    edge_index = np.asarray(inputs["edge_index"], np.int32)
    nc, srcT, dstlocT, dstownT = _get_program(edge_index.tobytes(), edge_index)

    import ml_dtypes
    bf = lambda a: np.asarray(a, np.float32).astype(ml_dtypes.bfloat16)

    W1e = _wext(np.asarray(inputs["W1"], np.float32), inputs["a1s"], inputs["a1d"])
    W2e = _wext(np.asarray(inputs["W2"], np.float32), inputs["a2s"], inputs["a2d"])
    W3e = _wext(np.asarray(inputs["W3"], np.float32), inputs["a3s"], inputs["a3d"])
    sc1, sh1 = _bn_cols(inputs["g1"], inputs["be1"], inputs["b1"])
    sc2, sh2 = _bn_cols(inputs["g2"], inputs["be2"], inputs["b2"])
    sc3, sh3 = _bn_cols(inputs["g3"], inputs["be3"], inputs["b3"])

    shared = {
        "W1e": bf(W1e), "W2e": bf(W2e), "W3e": bf(W3e),
        "sc1": sc1, "sh1": sh1, "sc2": sc2, "sh2": sh2,
        "sc3": sc3, "sh3": sh3,
        "Wc1": np.asarray(inputs["Wc1"], np.float32),
        "bc1": np.asarray(inputs["bc1"], np.float32).reshape(-1, 1),
        "Wc2": np.asarray(inputs["Wc2"], np.float32),
        "bc2": np.asarray(inputs["bc2"], np.float32).reshape(-1, 1),
    }
    npc = N // NCORES
    in_maps = []
    for k in range(NCORES):
        m = dict(shared)
        m["xT"] = bf(np.ascontiguousarray(x[k * npc:(k + 1) * npc].T))
        m["srcT"] = srcT[k]
        m["dstlocT"] = dstlocT[k]
        m["dstownT"] = dstownT[k]
        in_maps.append(m)
    return nc, in_maps


def kernel(**inputs):
    nc, in_maps = prepare(inputs)
    res = bass_utils.run_bass_kernel_spmd(nc, in_maps, core_ids=list(range(NCORES)))
    outs = [res.results[k]["out"] for k in range(NCORES)]
    return np.concatenate(outs, axis=0).astype(np.float32)

